# revision 1
# baseline (speedup 1.0000x reference)
"""Trainium2 Bass kernel for nn_BfpQuantizer: block-floating-point
quantizer (qtorch-style float_quantize to 8-exp/7-man float == bf16 RNE,
then 8-wide shared-exponent block quantize, wl=8).

Contract: kernel(x) takes the FULL fp32 input (8, 2048, 4096) and returns
the FULL output, bit-exact with the exact-math semantics of the reference:
  fq  = bf16_rne(x)                       (== float_quantize(x, 8, 7))
  M   = max |fq| over each block of 8 (last axis)
  e   = floor(log2(M)); scale = 2^(e-6)
  out = clip(round_rne(fq/scale), -127, 127) * scale

Sharding: fully data-parallel -- batch dim 8 maps 1:1 onto the 8
NeuronCores; no cross-device communication.

Per-core pipeline (one tile = 128 partitions x 2048 fp32 elements, all
HBM DMAs are single contiguous runs):
  ACT : fq  = bf16(x)        (copy, RNE)       -- contiguous
        afq = bf16(|x|)      (Abs activation)  -- contiguous
        y   = fp32(obf)      (copy, exact)     -- contiguous
  DVE : M via 3-op max tree over afq (blocks along free axis)
        per-block scale/inv bits in int16 on the bf16 bit pattern:
          tb   = (bits(M) >> 7) << 7          biased-exponent field
          invb = 33280 - tb  == bits of 2^(6-e)   [saturation-safe form:
                 (~(tb - 16640)) + 16641 -- the HW int16 ALU saturates]
          sclb = tb - 768    == bits of 2^(e-6)
        (per-block tensors are built pair-duplicated [P, G, 2] so the two
         multiplies read them through an innermost-contiguous broadcast AP
         [g][0,4][1,2], which keeps the DVE in its 2x perf mode)
        p   = fq * inv               (exact in bf16)
        pc  = clip(p, +-127.25)      (folds the +-127.5 -> +-128 case into
                                      the later clip at +-127; 127.25 is
                                      exactly halfway between bf16 values
                                      so no other p is affected)
        r   = (pc + 1.5*2^23) - 1.5*2^23   (fp32-ALU RNE round-to-int)
        obf = r * scl                (exact in bf16)
No collectives, no transposes, no broadcast DMA traffic.
"""
import sys

sys.path.insert(0, "/opt/trn_rl_repo")

import numpy as np

import concourse.bass as bass
import concourse.tile as tile
from concourse import mybir

MAGIC = 12582912.0  # 1.5 * 2**23
N_CORES = 8
ROWS, COLS = 2048, 4096  # per-core shard (full input is (8, 2048, 4096))


def _fix_waits(nc):
    """walrus in this container encodes at most 1 sync wait per
    instruction (2 for InstEventSemaphore); Tile attaches more. Hoist the
    excess waits onto standalone NoOps just before the instruction."""
    for blk in nc.m.functions[0].blocks:
        new = []
        for inst in blk.instructions:
            si = inst.sync_info
            cap = 2 if isinstance(inst, mybir.InstEventSemaphore) else 1
            if si is not None and si.on_wait and len(si.on_wait) > cap:
                waits = list(si.on_wait)
                excess, keep = waits[:-cap], waits[-cap:]
                for k, w in enumerate(excess):
                    new.append(mybir.InstNoOp(
                        name=f"{inst.name}-hw{k}",
                        engine=inst.engine,
                        sync_info=mybir.SyncInfo(on_wait=[w], on_update=[]),
                    ))
                si.on_wait = keep
            new.append(inst)
        blk.instructions = new
    return nc


def build_nc(rows=ROWS, cols=COLS, tile_free=2048, bufs=3):
    P = 128
    TF = tile_free
    G = TF // 8
    ntiles = rows * cols // (P * TF)
    assert ntiles * P * TF == rows * cols
    A = mybir.AluOpType

    nc = bass.Bass()
    x = nc.dram_tensor("x", [rows, cols], mybir.dt.float32, kind="ExternalInput")
    y = nc.dram_tensor("y", [rows, cols], mybir.dt.float32, kind="ExternalOutput")
    xv = x.rearrange("r c -> (r c)").rearrange("(t p f) -> t p f", p=P, f=TF)
    yv = y.rearrange("r c -> (r c)").rearrange("(t p f) -> t p f", p=P, f=TF)

    with tile.TileContext(nc) as tc:
        with tc.tile_pool(name="pool", bufs=bufs) as pool:
            for t in range(ntiles):
                xt = pool.tile([P, TF], mybir.dt.float32, tag="xt")
                nc.sync.dma_start(out=xt, in_=xv[t])
                fq = pool.tile([P, G, 8], mybir.dt.bfloat16, tag="fq")
                nc.scalar.copy(fq.rearrange("p g b -> p (g b)"), xt)
                afq = pool.tile([P, G, 8], mybir.dt.bfloat16, tag="afq")
                nc.scalar.activation(afq.rearrange("p g b -> p (g b)"), xt,
                                     mybir.ActivationFunctionType.Abs)
                s1 = pool.tile([P, G, 4], mybir.dt.bfloat16, tag="s1")
                nc.vector.tensor_tensor(s1, afq[:, :, 0:4], afq[:, :, 4:8], A.max)
                s2 = pool.tile([P, G, 2], mybir.dt.bfloat16, tag="s2")
                nc.vector.tensor_tensor(s2, s1[:, :, 0:2], s1[:, :, 2:4], A.max)
                M2 = pool.tile([P, G, 2], mybir.dt.bfloat16, tag="M2")
                nc.vector.tensor_tensor(M2[:, :, 0], s2[:, :, 0], s2[:, :, 1], A.max)
                nc.vector.tensor_tensor(M2[:, :, 1], s2[:, :, 0], s2[:, :, 1], A.max)
                M2f = M2.rearrange("p g b -> p (g b)")
                tb = pool.tile([P, G, 2], mybir.dt.int16, tag="tb")
                tbf = tb.rearrange("p g b -> p (g b)")
                nc.vector.tensor_scalar(tbf, M2f.bitcast(mybir.dt.int16), 7, 7,
                                        A.logical_shift_right, A.logical_shift_left)
                t2 = pool.tile([P, G, 2], mybir.dt.int16, tag="t2")
                t2f = t2.rearrange("p g b -> p (g b)")
                nc.vector.tensor_scalar(t2f, tbf, 16640, None, A.subtract)
                t3 = pool.tile([P, G, 2], mybir.dt.int16, tag="t3")
                t3f = t3.rearrange("p g b -> p (g b)")
                nc.vector.tensor_scalar(t3f, t2f, 0, None, A.bitwise_not)
                invb = pool.tile([P, G, 2], mybir.dt.int16, tag="invb")
                nc.vector.tensor_scalar(invb.rearrange("p g b -> p (g b)"), t3f,
                                        16641, None, A.add)
                sclb = pool.tile([P, G, 2], mybir.dt.int16, tag="sclb")
                nc.vector.tensor_scalar(sclb.rearrange("p g b -> p (g b)"), tbf,
                                        768, None, A.subtract)
                inv2 = invb.bitcast(mybir.dt.bfloat16)
                scl2 = sclb.bitcast(mybir.dt.bfloat16)
                inv_b = inv2.unsqueeze(2).broadcast_to((P, G, 4, 2))
                scl_b = scl2.unsqueeze(2).broadcast_to((P, G, 4, 2))
                fq4 = fq.rearrange("p g (c b) -> p g c b", b=2)
                p_t = pool.tile([P, G, 4, 2], mybir.dt.bfloat16, tag="p")
                nc.vector.tensor_tensor(p_t, fq4, inv_b, A.mult)
                pf = p_t.rearrange("p g c b -> p (g c b)")
                pc = pool.tile([P, TF], mybir.dt.bfloat16, tag="pc")
                nc.vector.tensor_scalar(pc, pf, 127.25, -127.25, A.min, A.max)
                r = pool.tile([P, TF], mybir.dt.bfloat16, tag="r")
                nc.vector.tensor_scalar(r, pc, MAGIC, MAGIC, A.add, A.subtract)
                obf = pool.tile([P, G, 4, 2], mybir.dt.bfloat16, tag="obf")
                nc.vector.tensor_tensor(obf,
                                        r.rearrange("p (g c b) -> p g c b", g=G, b=2),
                                        scl_b, A.mult)
                yt = pool.tile([P, TF], mybir.dt.float32, tag="yt")
                nc.scalar.copy(yt, obf.rearrange("p g c b -> p (g c b)"))
                nc.sync.dma_start(out=yv[t], in_=yt)
    _fix_waits(nc)
    return nc


_CACHED_NC = None


def _get_nc():
    global _CACHED_NC
    if _CACHED_NC is None:
        _CACHED_NC = build_nc()
    return _CACHED_NC


def kernel(x: np.ndarray) -> np.ndarray:
    """Full-input entry point: x (8, 2048, 4096) fp32 -> same-shape fp32."""
    from concourse.bass_utils import run_bass_kernel_spmd

    x = np.ascontiguousarray(np.asarray(x, dtype=np.float32))
    assert x.shape == (N_CORES, ROWS, COLS), x.shape
    nc = _get_nc()
    in_maps = [{"x": x[i]} for i in range(N_CORES)]
    res = run_bass_kernel_spmd(nc, in_maps, list(range(N_CORES)))
    out = np.stack([res.results[i]["y"] for i in range(N_CORES)])
    return out.astype(np.float32, copy=False)



# revision 2
# speedup vs baseline: 1.0733x; 1.0733x over previous
"""Trainium2 Bass kernel v4 for nn_BfpQuantizer.

Math (bit-exact with the exact-math semantics of the reference):
  fq  = bf16_rne(x)                    (== float_quantize(x, 8, 7))
  M   = max |fq| over each block of 8 (last axis)
  e   = floor(log2(M))
  out = clip(rne(fq * 2^-e), +-1.984375) * 2^e   [grid 2^-6, then rescale]

v4 changes vs v3 (188.8us):
  - normalized domain 2^-e instead of 2^(6-e): the scale bits are just
    M2bits & 0x7F80 (one tensor_scalar, no shift) and the inverse is
    32512 - tb (saturation-free); drops one DVE inst per tile and makes
    all-zero blocks exact instead of relying on their absence
  - TF=8192: halves DVE instruction count (per-inst overhead ~150ns)
  - SBUF reuse: M2 and tb live in s1's buffer, pc/r in-place on p,
    obf reuses afq -> 64KB/buf, bufs=3

Engine budget per core (measured rates): DVE ~156us (bound),
DMA ~144us, ACT ~59us.
"""
import sys

sys.path.insert(0, "/opt/trn_rl_repo")

import numpy as np

import concourse.bass as bass
import concourse.tile as tile
from concourse import mybir

MAGIC = 196608.0      # 1.5 * 2**17: fp32 RNE to multiples of 2^-6
CLIP = 1.98828125     # 127.25/64, folds the +-127.5->128 case into 127
N_CORES = 8
ROWS, COLS = 2048, 4096  # per-core shard (full input is (8, 2048, 4096))


def _fix_waits(nc):
    """walrus in this container encodes at most 1 sync wait per
    instruction (2 for InstEventSemaphore); Tile attaches more. Hoist the
    excess waits onto standalone NoOps just before the instruction."""
    for blk in nc.m.functions[0].blocks:
        new = []
        for inst in blk.instructions:
            si = inst.sync_info
            cap = 2 if isinstance(inst, mybir.InstEventSemaphore) else 1
            if si is not None and si.on_wait and len(si.on_wait) > cap:
                waits = list(si.on_wait)
                excess, keep = waits[:-cap], waits[-cap:]
                for k, w in enumerate(excess):
                    new.append(mybir.InstNoOp(
                        name=f"{inst.name}-hw{k}",
                        engine=inst.engine,
                        sync_info=mybir.SyncInfo(on_wait=[w], on_update=[]),
                    ))
                si.on_wait = keep
            new.append(inst)
        blk.instructions = new
    return nc


def build_nc(rows=ROWS, cols=COLS, tile_free=8192, bufs=3, fix_waits=True):
    P = 128
    TF = tile_free
    G = TF // 8
    ntiles = rows * cols // (P * TF)
    assert ntiles * P * TF == rows * cols
    A = mybir.AluOpType

    nc = bass.Bass()
    x = nc.dram_tensor("x", [rows, cols], mybir.dt.float32, kind="ExternalInput")
    y = nc.dram_tensor("y", [rows, cols], mybir.dt.bfloat16, kind="ExternalOutput")
    xv = x.rearrange("r c -> (r c)").rearrange("(t p f) -> t p f", p=P, f=TF)
    yv = y.rearrange("r c -> (r c)").rearrange("(t p f) -> t p f", p=P, f=TF)

    with tile.TileContext(nc) as tc:
        with tc.tile_pool(name="pool", bufs=bufs) as pool:
            for t in range(ntiles):
                # fq = bf16(x): the SWDGE (gpsimd) input DMA casts
                # fp32->bf16 (RNE) during the transfer -- no ACT pass.
                fq = pool.tile([P, G, 8], mybir.dt.bfloat16, tag="fq")
                nc.gpsimd.dma_start(out=fq.rearrange("p g b -> p (g b)"), in_=xv[t])
                # afq = |fq|  (ACT, its only op)
                afq = pool.tile([P, G, 8], mybir.dt.bfloat16, tag="afq")
                nc.scalar.activation(afq.rearrange("p g b -> p (g b)"),
                                     fq.rearrange("p g b -> p (g b)"),
                                     mybir.ActivationFunctionType.Abs)
                # block max tree (DVE, all 2x); M2/tb live in s1's buffer
                s1 = pool.tile([P, G, 4], mybir.dt.bfloat16, tag="s1")
                nc.vector.tensor_tensor(s1, afq[:, :, 0:4], afq[:, :, 4:8], A.max)
                s2 = pool.tile([P, G, 2], mybir.dt.bfloat16, tag="s2")
                nc.vector.tensor_tensor(s2, s1[:, :, 0:2], s1[:, :, 2:4], A.max)
                M2 = s1[:, :, 0:2]  # pair-duplicated block max
                nc.vector.tensor_tensor(M2, s2, s2[:, :, ::-1], A.max)
                # tb   = bits(M) & 0x7F80 == bits of 2^e  (the scale)
                # invb = 32512 - tb       == bits of 2^-e (saturation-free)
                s1i = s1.bitcast(mybir.dt.int16)
                tb = s1i[:, :, 2:4]
                nc.vector.tensor_scalar(tb, s1i[:, :, 0:2], 0x7F80, None,
                                        A.bitwise_and)
                invb = pool.tile([P, G, 2], mybir.dt.int16, tag="invb")
                nc.vector.tensor_scalar(invb, tb, 32512, -1, A.subtract, A.mult)
                scl_b = s1.bitcast(mybir.dt.bfloat16)[:, :, 2:4] \
                    .unsqueeze(2).broadcast_to((P, G, 4, 2))
                inv_b = invb.bitcast(mybir.dt.bfloat16).unsqueeze(2) \
                    .broadcast_to((P, G, 4, 2))
                fq4 = fq.rearrange("p g (c b) -> p g c b", b=2)
                # p = fq * 2^-e; clip; RNE to grid 2^-6 via fp32 magic; * 2^e
                p_t = pool.tile([P, G, 4, 2], mybir.dt.bfloat16, tag="p")
                nc.vector.tensor_tensor(p_t, fq4, inv_b, A.mult)
                pf = p_t.rearrange("p g c b -> p (g c b)")
                nc.vector.tensor_scalar(pf, pf, CLIP, -CLIP, A.min, A.max)
                nc.vector.tensor_scalar(pf, pf, MAGIC, MAGIC, A.add, A.subtract)
                obf = afq  # afq is dead after s1; reuse its buffer
                nc.vector.tensor_tensor(obf.rearrange("p g (c b) -> p g c b", b=2),
                                        p_t, scl_b, A.mult)
                nc.sync.dma_start(out=yv[t], in_=obf.rearrange("p g b -> p (g b)"))
    if fix_waits:
        _fix_waits(nc)
    return nc


_CACHED_NC = None


def _get_nc():
    global _CACHED_NC
    if _CACHED_NC is None:
        _CACHED_NC = build_nc()
    return _CACHED_NC


def _bf16_to_f32(a: np.ndarray) -> np.ndarray:
    """Exact widening of a bf16 (or uint16-bit-pattern) array to fp32."""
    u = np.ascontiguousarray(a).view(np.uint16).astype(np.uint32) << 16
    return u.view(np.float32)


def kernel(x: np.ndarray) -> np.ndarray:
    """Full-input entry point: x (8, 2048, 4096) fp32 -> same-shape fp32."""
    from concourse.bass_utils import run_bass_kernel_spmd

    x = np.ascontiguousarray(np.asarray(x, dtype=np.float32))
    assert x.shape == (N_CORES, ROWS, COLS), x.shape
    nc = _get_nc()
    in_maps = [{"x": x[i]} for i in range(N_CORES)]
    res = run_bass_kernel_spmd(nc, in_maps, list(range(N_CORES)))
    out = np.stack([_bf16_to_f32(np.asarray(res.results[i]["y"])) for i in range(N_CORES)])
    return out.astype(np.float32, copy=False)


# revision 3
# speedup vs baseline: 1.1071x; 1.0315x over previous
"""Trainium2 Bass kernel v4 for nn_BfpQuantizer.

Math (bit-exact with the exact-math semantics of the reference):
  fq  = bf16_rne(x)                    (== float_quantize(x, 8, 7))
  M   = max |fq| over each block of 8 (last axis)
  e   = floor(log2(M))
  out = clip(rne(fq * 2^-e), +-1.984375) * 2^e   [grid 2^-6, then rescale]

v4 changes vs v3 (188.8us):
  - normalized domain 2^-e instead of 2^(6-e): the scale bits are just
    M2bits & 0x7F80 (one tensor_scalar, no shift) and the inverse is
    32512 - tb (saturation-free); drops one DVE inst per tile and makes
    all-zero blocks exact instead of relying on their absence
  - TF=8192: halves DVE instruction count (per-inst overhead ~150ns)
  - SBUF reuse: M2 and tb live in s1's buffer, pc/r in-place on p,
    obf reuses afq -> 64KB/buf, bufs=3

Engine budget per core (measured rates): DVE ~156us (bound),
DMA ~144us, ACT ~59us.
"""
import sys

sys.path.insert(0, "/opt/trn_rl_repo")

import numpy as np

import concourse.bass as bass
import concourse.tile as tile
from concourse import mybir

MAGIC = 196608.0      # 1.5 * 2**17: fp32 RNE to multiples of 2^-6
CLIP = 1.98828125     # 127.25/64, folds the +-127.5->128 case into 127
N_CORES = 8
ROWS, COLS = 2048, 4096  # per-core shard (full input is (8, 2048, 4096))


def _fix_waits(nc):
    """walrus in this container encodes at most 1 sync wait per
    instruction (2 for InstEventSemaphore); Tile attaches more. Hoist the
    excess waits onto standalone NoOps just before the instruction."""
    for blk in nc.m.functions[0].blocks:
        new = []
        for inst in blk.instructions:
            si = inst.sync_info
            cap = 2 if isinstance(inst, mybir.InstEventSemaphore) else 1
            if si is not None and si.on_wait and len(si.on_wait) > cap:
                waits = list(si.on_wait)
                excess, keep = waits[:-cap], waits[-cap:]
                for k, w in enumerate(excess):
                    new.append(mybir.InstNoOp(
                        name=f"{inst.name}-hw{k}",
                        engine=inst.engine,
                        sync_info=mybir.SyncInfo(on_wait=[w], on_update=[]),
                    ))
                si.on_wait = keep
            new.append(inst)
        blk.instructions = new
    return nc


def build_nc(rows=ROWS, cols=COLS, bufs=2, fix_waits=True):
    P = 128
    N = rows * cols // P  # elements per partition row
    # small lead-in/tail tiles shorten ramp and drain; 8192-wide main
    # tiles amortize per-instruction overheads. All tiles draw
    # full-size buffers from the same rings (sliced views).
    if N >= 16384 and N % 8192 == 0:
        tiles = [2048, 4096] + [8192] * (N // 8192 - 1) + [2048]
    else:
        tiles = [4096] * (N // 4096)
    assert sum(tiles) == N, (tiles, N)
    GMAX = max(tiles) // 8
    A = mybir.AluOpType

    nc = bass.Bass()
    x = nc.dram_tensor("x", [rows, cols], mybir.dt.float32, kind="ExternalInput")
    y = nc.dram_tensor("y", [rows, cols], mybir.dt.bfloat16, kind="ExternalOutput")
    xf = x.rearrange("r c -> (r c)").rearrange("(p n) -> p n", p=P)
    yf = y.rearrange("r c -> (r c)").rearrange("(p n) -> p n", p=P)

    with tile.TileContext(nc) as tc:
        with tc.tile_pool(name="pool", bufs=bufs) as pool:
            off = 0
            for TF in tiles:
                G = TF // 8
                xv_t = xf[:, off:off + TF]
                yv_t = yf[:, off:off + TF]
                off += TF
                # fp32 input over HWDGE (fast path), split in two
                # half-tile DMAs so ACT can start converting early;
                # ACT does fq = bf16(x) and afq = bf16(|x|) per half
                xt = pool.tile([P, 8 * GMAX], mybir.dt.float32, tag="xt", name="xt")[:, :TF]
                h = TF // 2
                nc.sync.dma_start(out=xt[:, 0:h], in_=xv_t[:, 0:h])
                nc.sync.dma_start(out=xt[:, h:TF], in_=xv_t[:, h:TF])
                fq = pool.tile([P, GMAX, 8], mybir.dt.bfloat16, tag="fq", name="fq")[:, :G]
                afq = pool.tile([P, GMAX, 8], mybir.dt.bfloat16, tag="afq", name="afq")[:, :G]
                fqf = fq.rearrange("p g b -> p (g b)")
                afqf = afq.rearrange("p g b -> p (g b)")
                nc.scalar.copy(fqf[:, 0:h], xt[:, 0:h])
                nc.scalar.activation(afqf[:, 0:h], xt[:, 0:h],
                                     mybir.ActivationFunctionType.Abs)
                nc.scalar.copy(fqf[:, h:TF], xt[:, h:TF])
                nc.scalar.activation(afqf[:, h:TF], xt[:, h:TF],
                                     mybir.ActivationFunctionType.Abs)
                # block max tree (DVE, all 2x); M2/tb live in s1's buffer
                s1 = pool.tile([P, GMAX, 4], mybir.dt.bfloat16, tag="s1", name="s1")[:, :G]
                nc.vector.tensor_tensor(s1, afq[:, :, 0:4], afq[:, :, 4:8], A.max)
                s2 = pool.tile([P, GMAX, 2], mybir.dt.bfloat16, tag="s2", name="s2")[:, :G]
                nc.vector.tensor_tensor(s2, s1[:, :, 0:2], s1[:, :, 2:4], A.max)
                M2 = s1[:, :, 0:2]  # pair-duplicated block max
                nc.vector.tensor_tensor(M2, s2, s2[:, :, ::-1], A.max)
                # tb   = bits(M) & 0x7F80 == bits of 2^e  (the scale)
                # invb = 32512 - tb       == bits of 2^-e (saturation-free)
                s1i = s1.bitcast(mybir.dt.int16)
                tb = s1i[:, :, 2:4]
                nc.vector.tensor_scalar(tb, s1i[:, :, 0:2], 0x7F80, None,
                                        A.bitwise_and)
                invb = pool.tile([P, GMAX, 2], mybir.dt.int16, tag="invb", name="invb")[:, :G]
                nc.vector.tensor_scalar(invb, tb, 32512, -1, A.subtract, A.mult)
                scl_b = s1.bitcast(mybir.dt.bfloat16)[:, :, 2:4] \
                    .unsqueeze(2).broadcast_to((P, G, 4, 2))
                inv_b = invb.bitcast(mybir.dt.bfloat16).unsqueeze(2) \
                    .broadcast_to((P, G, 4, 2))
                fq4 = fq.rearrange("p g (c b) -> p g c b", b=2)
                # p = fq * 2^-e; clip; RNE to grid 2^-6 via fp32 magic; * 2^e
                p_t = pool.tile([P, GMAX, 4, 2], mybir.dt.bfloat16, tag="p", name="p")[:, :G]
                nc.vector.tensor_tensor(p_t, fq4, inv_b, A.mult)
                pf = p_t.rearrange("p g c b -> p (g c b)")
                nc.vector.tensor_scalar(pf, pf, CLIP, -CLIP, A.min, A.max)
                nc.vector.tensor_scalar(pf, pf, MAGIC, MAGIC, A.add, A.subtract)
                # obf in place over p (afq must free early, at s1)
                nc.vector.tensor_tensor(p_t, p_t, scl_b, A.mult)
                nc.sync.dma_start(out=yv_t, in_=p_t.rearrange("p g c b -> p (g c b)"))
    if fix_waits:
        _fix_waits(nc)
    return nc


_CACHED_NC = None


def _get_nc():
    global _CACHED_NC
    if _CACHED_NC is None:
        _CACHED_NC = build_nc()
    return _CACHED_NC


def _bf16_to_f32(a: np.ndarray) -> np.ndarray:
    """Exact widening of a bf16 (or uint16-bit-pattern) array to fp32."""
    u = np.ascontiguousarray(a).view(np.uint16).astype(np.uint32) << 16
    return u.view(np.float32)


def kernel(x: np.ndarray) -> np.ndarray:
    """Full-input entry point: x (8, 2048, 4096) fp32 -> same-shape fp32."""
    from concourse.bass_utils import run_bass_kernel_spmd

    x = np.ascontiguousarray(np.asarray(x, dtype=np.float32))
    assert x.shape == (N_CORES, ROWS, COLS), x.shape
    nc = _get_nc()
    in_maps = [{"x": x[i]} for i in range(N_CORES)]
    res = run_bass_kernel_spmd(nc, in_maps, list(range(N_CORES)))
    out = np.stack([_bf16_to_f32(np.asarray(res.results[i]["y"])) for i in range(N_CORES)])
    return out.astype(np.float32, copy=False)
